# revision 18
# baseline (speedup 1.0000x reference)
"""Viterbi CRF decode on 8 Trainium2 NeuronCores — exp-domain PE formulation.

Strategy: data-parallel over batch (32 sequences/core). The forward max-plus DP
runs in the exponential domain so the TensorEngine does the heavy lifting:

    EZ_t[j,b]   = exp(K*(alpha_t[b,j] - n_t[b]))       (n_t arbitrary per-(b,t))
    V[j,b]      = sum_i expW[i,j] * EZ_{t-1}[i,b]      (PE matmul, bf16 in/fp32 acc)
    EZ_t        = V * EP_t                             (DVE elementwise, one op)

where expW = exp(K*trans) and EP_t[j,b] = exp(K*(pot[b,t,j]-max_j pot[b,t,j]))
are host-precomputed input transforms. logsumexp/K approximates max within
log(64)/K; with K=128 the measured error is ~34/262144 tag flips (rel ~1e-4
vs the 2e-2 gate). bf16 quantization of EZ/W/EP adds ~6e-5 alpha noise per
step (products of bf16 are exact in the fp32 PSUM accumulate).

The backtrack only compares values within one (b,t) slice, so any per-(b,t)
rescale of EZ is harmless: argmax_i(alpha[i]+trans[i,j]) == argmax_i
EZ[i]*expW[i,j] (monotone). Every G steps a 1/sum_i EZ rescale (per stream) is
folded off-critical-path into the EP slice of step t+LAG, so the serial chain
is structurally identical every step: one matmul + one DVE multiply.

Device layout: 32 sequences split into two 16-seq streams on partitions 0-63 /
64-127 with BLOCK-DIAGONAL weights [128,128], so ONE matmul per step serves
both streams (K=M=128, N=16), writing one PSUM bank that a single [128,16]
DVE multiply turns into the next bf16 state, written straight into the
history buffer (also the next matmul's rhs). The S-row sums use a [128,2]
selector matmul; the per-b broadcast of 1/S uses a K=2 mask matmul. A one-time
burst of dummy matmuls at startup locks the PE HAM clock-gate at 2.4 GHz
(steady-state gaps are too short to ever re-throttle it).

History streams back to HBM per 128-step chunk; the host backtracks in f64.
"""

import numpy as np

B, L, T = 256, 1024, 64
NCORES = 8
BC = B // NCORES   # 32 sequences per core
HB = BC // 2       # 16 sequences per stream
CH = 128           # steps per DMA chunk
KSC = 128.0        # exp-domain scale
G = 16             # renormalize every G steps
LAG = 4            # renorm scale applied LAG steps after it is measured
NWARM = 40         # startup dummy matmuls to warm the PE HAM clock gate

_cache = {}


def _build_program():
    if "nc" in _cache:
        return _cache["nc"]
    import concourse.bacc as bacc
    import concourse.mybir as mybir
    from concourse.tile import TileContext

    f32 = mybir.dt.float32
    bf16 = mybir.dt.bfloat16

    nc = bacc.Bacc("TRN2", target_bir_lowering=False, debug=False)
    ep_in = nc.dram_tensor("ep", [128, L, HB], bf16, kind="ExternalInput").ap()
    w_in = nc.dram_tensor("wbig", [128, 128], bf16, kind="ExternalInput").ap()
    scol_in = nc.dram_tensor("scol", [128, 2], bf16, kind="ExternalInput").ap()
    bmask_in = nc.dram_tensor("bmask", [2, 128], bf16, kind="ExternalInput").ap()
    hist_out = nc.dram_tensor("ezhist", [128, L, HB], bf16, kind="ExternalOutput").ap()

    with TileContext(nc) as tc:
        with tc.tile_pool(name="const", bufs=1) as cpool, \
             tc.tile_pool(name="ep", bufs=2) as eppool, \
             tc.tile_pool(name="hist", bufs=1) as hpool, \
             tc.tile_pool(name="psv", bufs=4, space="PSUM") as vpool, \
             tc.tile_pool(name="pss", bufs=1, space="PSUM") as spool, \
             tc.tile_pool(name="warm", bufs=1, space="PSUM") as wpool:
            wbig = cpool.tile([128, 128], bf16)
            nc.gpsimd.dma_start(out=wbig[:], in_=w_in[:])
            scol = cpool.tile([128, 2], bf16)
            nc.gpsimd.dma_start(out=scol[:], in_=scol_in[:])
            bmask = cpool.tile([2, 128], bf16)
            nc.gpsimd.dma_start(out=bmask[:], in_=bmask_in[:])
            rs32 = cpool.tile([2, HB], f32)
            rsb = cpool.tile([2, HB], bf16)
            eps1 = cpool.tile([128, HB], bf16)
            hist = hpool.tile([128, L, HB], bf16)   # 32KB/partition

            # One-time PE warmup: ~NWARM back-to-back dummy matmuls (~4-5us of
            # continuous PE activity) flip the HAM clock gate to K=8/8; the
            # per-step gaps afterwards are far below the ~3.4us idle window,
            # so it never re-throttles. A memset source avoids any DMA
            # dependency, so warmup overlaps the input DMAs.
            wsrc = cpool.tile([128, 64], bf16)
            nc.any.memset(wsrc[:], 1.0)
            warm = wpool.tile([64, 64], f32)
            for _ in range(NWARM):
                nc.tensor.matmul(warm[:], wsrc[:], wsrc[:], start=True, stop=True)

            nchunks = L // CH
            for c in range(nchunks):
                ep = eppool.tile([128, CH, HB], bf16, tag="ep")
                nc.gpsimd.dma_start(out=ep[:], in_=ep_in[:, c * CH:(c + 1) * CH, :])

                if c == 0:
                    nc.scalar.copy(hist[:, 0, :], ep[:, 0, :])

                t0 = max(c * CH, 1)
                # Deferred kick tails: emitted 1 / 3 steps after the kick so
                # each op lands in an engine idle gap instead of stalling the
                # chain in the strict per-engine FIFOs.
                recip_at = {}   # t -> sp psum tile
                apply_at = {}   # t -> target ep slot for eps1
                for t in range(t0, (c + 1) * CH):
                    s = t - c * CH
                    vp = vpool.tile([128, HB], f32, tag="v")
                    nc.tensor.matmul(vp[0:64, :], wbig[0:64, 0:64],
                                     hist[0:64, t - 1, :], start=True, stop=True)
                    nc.tensor.matmul(vp[64:128, :], wbig[64:128, 64:128],
                                     hist[64:128, t - 1, :], start=True, stop=True)
                    # The renorm scale is pre-folded into eps1 for apply steps,
                    # so the serial chain is identical every step.
                    use_eps1 = t % G == LAG and t >= G + LAG
                    src1 = eps1[:] if use_eps1 else ep[:, s, :]
                    nc.vector.tensor_mul(hist[:, t, :], vp[:], src1)
                    if t in recip_at:
                        sp = recip_at.pop(t)
                        nc.vector.reciprocal(rs32[:], sp[:])
                        nc.scalar.copy(rsb[:], rs32[:])
                    if t in apply_at:
                        tgt = apply_at.pop(t)
                        bcp = spool.tile([128, HB], f32, tag="bc")
                        nc.tensor.matmul(bcp[:], bmask[:], rsb[:],
                                         start=True, stop=True)
                        nc.vector.tensor_mul(eps1[:], ep[:, tgt, :], bcp[:])
                    if t % G == 0 and G <= t < L - LAG:
                        # Off-chain renorm kick: S = per-stream sum_i EZ_t
                        # (rows 0/1 via the selector matmul); 1/S is broadcast
                        # with a K=2 mask matmul and folded into the EP slice
                        # of step t+LAG (always within the same chunk).
                        sp = spool.tile([2, HB], f32, tag="s")
                        nc.tensor.matmul(sp[:], scol[:], hist[:, t, :],
                                         start=True, stop=True)
                        recip_at[t + 1] = sp
                        apply_at[t + 3] = s + LAG

                nc.gpsimd.dma_start(
                    out=hist_out[:, c * CH:(c + 1) * CH, :],
                    in_=hist[:, c * CH:(c + 1) * CH, :],
                )

    nc.compile()
    _cache["nc"] = nc
    return nc


def _host_precompute(potentials, trans):
    """Per-core EP in [128, L, 16] bf16 stream layout + block-diag weights."""
    import ml_dtypes
    bf = ml_dtypes.bfloat16
    pm = potentials.max(axis=2, keepdims=True)
    EP = np.exp(KSC * (potentials - pm).astype(np.float64)).astype(bf)  # [B,L,T]
    eps = []
    for c in range(NCORES):
        epc = EP[c * BC:(c + 1) * BC]                    # [32, L, T]
        h = epc.reshape(2, HB, L, T).transpose(0, 3, 2, 1)  # [2, T, L, HB]
        eps.append(np.ascontiguousarray(h.reshape(128, L, HB)))
    expW = np.exp(KSC * trans.astype(np.float64)).astype(np.float32)    # [i, j]
    wbig = np.zeros((128, 128), dtype=bf)
    wbig[0:T, 0:T] = expW.astype(bf)
    wbig[T:128, T:128] = expW.astype(bf)
    scol = np.zeros((128, 2), dtype=bf)
    scol[0:T, 0] = 1
    scol[T:128, 1] = 1
    bmask = np.zeros((2, 128), dtype=bf)
    bmask[0, 0:T] = 1
    bmask[1, T:128] = 1
    return eps, wbig, scol, bmask, expW


def kernel(potentials, lengths, transition_params):
    from concourse.bass_utils import run_bass_kernel_spmd

    potentials = np.ascontiguousarray(np.asarray(potentials, dtype=np.float32))
    lengths = np.asarray(lengths, dtype=np.int32)
    trans = np.ascontiguousarray(np.asarray(transition_params, dtype=np.float32))

    nc = _build_program()
    eps, wbig, scol, bmask, expW = _host_precompute(potentials, trans)
    in_maps = [{"ep": eps[c], "wbig": wbig, "scol": scol, "bmask": bmask}
               for c in range(NCORES)]
    res = run_bass_kernel_spmd(nc, in_maps, core_ids=list(range(NCORES)))
    # [128, L, HB] per core -> EZ [B, L, T]
    parts = []
    for c in range(NCORES):
        arr = res.results[c]["ezhist"]                       # [128, L, HB] bf16
        h = arr.reshape(2, T, L, HB).transpose(0, 3, 2, 1)   # [2, HB, L, T]
        parts.append(h.reshape(BC, L, T))
    EZ = np.concatenate(parts, axis=0).astype(np.float64)    # [B, L, T]

    # Host backtrack in exp domain (monotone-equivalent to max-plus argmax).
    tags = np.zeros((B, L), dtype=np.int64)
    last = EZ[np.arange(B), lengths - 1, :].argmax(axis=1)
    tags[:, L - 1] = last
    lm1 = lengths - 1
    EW = expW.astype(np.float64)
    for t in range(L - 2, -1, -1):
        nxt = tags[:, t + 1]
        cand = EZ[:, t, :] * EW[:, nxt].T
        tags[:, t] = np.where(t >= lm1, last, cand.argmax(axis=1))
    return tags.astype(np.int32)


# revision 20
# speedup vs baseline: 1.3920x; 1.3920x over previous
"""Viterbi CRF decode on 8 Trainium2 NeuronCores — exp-domain PE formulation.

Strategy: data-parallel over batch (32 sequences/core). The forward max-plus DP
runs in the exponential domain so the TensorEngine does the heavy lifting:

    EZ_t[j,b]   = exp(K*(alpha_t[b,j] - n_t[b]))       (n_t arbitrary per-(b,t))
    V[j,b]      = sum_i expW[i,j] * EZ_{t-1}[i,b]      (PE matmul, bf16 in/fp32 acc)
    EZ_t        = V * EP_t                             (DVE elementwise, one op)

where expW = exp(K*trans) and EP_t[j,b] = exp(K*(pot[b,t,j]-max_j pot[b,t,j]))
are host-precomputed input transforms. logsumexp/K approximates max within
log(64)/K; with K=128 the measured error is ~34/262144 tag flips (rel ~1e-4
vs the 2e-2 gate). bf16 quantization of EZ/W/EP adds ~6e-5 alpha noise per
step (products of bf16 are exact in the fp32 PSUM accumulate).

The backtrack only compares values within one (b,t) slice, so any per-(b,t)
rescale of EZ is harmless: argmax_i(alpha[i]+trans[i,j]) == argmax_i
EZ[i]*expW[i,j] (monotone). Every G steps a 1/sum_i EZ rescale (per stream) is
folded off-critical-path into the EP slice of step t+LAG, so the serial chain
is structurally identical every step: one matmul + one DVE multiply.

Device layout: 32 sequences split into two 16-seq streams on partitions 0-63 /
64-127 with BLOCK-DIAGONAL weights [128,128], so ONE matmul per step serves
both streams (K=M=128, N=16), writing one PSUM bank that a single [128,16]
DVE multiply turns into the next bf16 state, written straight into the
history buffer (also the next matmul's rhs). The S-row sums use a [128,2]
selector matmul; the per-b broadcast of 1/S uses a K=2 mask matmul. A one-time
burst of dummy matmuls at startup locks the PE HAM clock-gate at 2.4 GHz
(steady-state gaps are too short to ever re-throttle it).

History streams back to HBM per 128-step chunk; the host backtracks in f64.
"""

import numpy as np

B, L, T = 256, 1024, 64
NCORES = 8
BC = B // NCORES   # 32 sequences per core
HB = BC // 2       # 16 sequences per stream
CH = 128           # steps per DMA chunk
KSC = 128.0        # exp-domain scale
G = 16             # renormalize every G steps
LAG = 4            # renorm scale applied LAG steps after it is measured
NWARM = 40         # startup dummy matmuls to warm the PE HAM clock gate

_cache = {}


def _build_program():
    if "nc" in _cache:
        return _cache["nc"]
    import concourse.bacc as bacc
    import concourse.mybir as mybir
    from concourse.tile import TileContext

    f32 = mybir.dt.float32
    bf16 = mybir.dt.bfloat16

    nc = bacc.Bacc("TRN2", target_bir_lowering=False, debug=False)
    ep_in = nc.dram_tensor("ep", [128, L, HB], bf16, kind="ExternalInput").ap()
    w_in = nc.dram_tensor("wbig", [128, 128], bf16, kind="ExternalInput").ap()
    scol_in = nc.dram_tensor("scol", [128, 2], bf16, kind="ExternalInput").ap()
    bmask_in = nc.dram_tensor("bmask", [2, 128], bf16, kind="ExternalInput").ap()
    hist_out = nc.dram_tensor("ezhist", [128, L, HB], bf16, kind="ExternalOutput").ap()

    with TileContext(nc) as tc:
        with tc.tile_pool(name="const", bufs=1) as cpool, \
             tc.tile_pool(name="ep", bufs=2) as eppool, \
             tc.tile_pool(name="hist", bufs=1) as hpool, \
             tc.tile_pool(name="psv", bufs=4, space="PSUM") as vpool, \
             tc.tile_pool(name="pss", bufs=1, space="PSUM") as spool, \
             tc.tile_pool(name="warm", bufs=1, space="PSUM") as wpool:
            wbig = cpool.tile([128, 128], bf16)
            nc.gpsimd.dma_start(out=wbig[:], in_=w_in[:])
            scol = cpool.tile([128, 2], bf16)
            nc.gpsimd.dma_start(out=scol[:], in_=scol_in[:])
            bmask = cpool.tile([2, 128], bf16)
            nc.gpsimd.dma_start(out=bmask[:], in_=bmask_in[:])
            rs32 = cpool.tile([2, HB], f32)
            rsb = cpool.tile([2, HB], bf16)
            eps1 = cpool.tile([128, HB], bf16)
            hist = hpool.tile([128, L, HB], bf16)   # 32KB/partition

            # One-time PE warmup: ~NWARM back-to-back dummy matmuls (~4-5us of
            # continuous PE activity) flip the HAM clock gate to K=8/8; the
            # per-step gaps afterwards are far below the ~3.4us idle window,
            # so it never re-throttles. A memset source avoids any DMA
            # dependency, so warmup overlaps the input DMAs.
            wsrc = cpool.tile([128, 64], bf16)
            nc.any.memset(wsrc[:], 1.0)
            warm = wpool.tile([64, 64], f32)
            for _ in range(NWARM):
                nc.tensor.matmul(warm[:], wsrc[:], wsrc[:], start=True, stop=True)

            nchunks = L // CH
            for c in range(nchunks):
                ep = eppool.tile([128, CH, HB], bf16, tag="ep")
                nc.gpsimd.dma_start(out=ep[:], in_=ep_in[:, c * CH:(c + 1) * CH, :])

                if c == 0:
                    nc.scalar.copy(hist[:, 0, :], ep[:, 0, :])

                t0 = max(c * CH, 1)
                # Deferred kick tails: emitted 1 / 3 steps after the kick so
                # each op lands in an engine idle gap instead of stalling the
                # chain in the strict per-engine FIFOs.
                recip_at = {}   # t -> sp psum tile
                apply_at = {}   # t -> target ep slot for eps1
                for t in range(t0, (c + 1) * CH):
                    s = t - c * CH
                    vp = vpool.tile([128, HB], f32, tag="v")
                    nc.tensor.matmul(vp[:], wbig[:], hist[:, t - 1, :],
                                     start=True, stop=True)
                    # The renorm scale is pre-folded into eps1 for apply steps,
                    # so the serial chain is identical every step.
                    use_eps1 = t % G == LAG and t >= G + LAG
                    src1 = eps1[:] if use_eps1 else ep[:, s, :]
                    nc.vector.tensor_mul(hist[:, t, :], vp[:], src1)
                    if t in recip_at:
                        sp = recip_at.pop(t)
                        with tc.high_priority(offset=-6):
                            nc.vector.reciprocal(rs32[:], sp[:])
                            nc.scalar.copy(rsb[:], rs32[:])
                    if t in apply_at:
                        tgt = apply_at.pop(t)
                        with tc.high_priority(offset=-3):
                            bcp = spool.tile([128, HB], f32, tag="bc")
                            nc.tensor.matmul(bcp[:], bmask[:], rsb[:],
                                             start=True, stop=True)
                            nc.vector.tensor_mul(eps1[:], ep[:, tgt, :], bcp[:])
                    if t % G == 0 and G <= t < L - LAG:
                        # Off-chain renorm kick: S = per-stream sum_i EZ_t
                        # (rows 0/1 via the selector matmul); 1/S is broadcast
                        # with a K=2 mask matmul and folded into the EP slice
                        # of step t+LAG (always within the same chunk).
                        sp = spool.tile([2, HB], f32, tag="s")
                        nc.tensor.matmul(sp[:], scol[:], hist[:, t, :],
                                         start=True, stop=True)
                        recip_at[t + 1] = sp
                        apply_at[t + 3] = s + LAG

                nc.gpsimd.dma_start(
                    out=hist_out[:, c * CH:(c + 1) * CH, :],
                    in_=hist[:, c * CH:(c + 1) * CH, :],
                )

    nc.compile()
    _cache["nc"] = nc
    return nc


def _host_precompute(potentials, trans):
    """Per-core EP in [128, L, 16] bf16 stream layout + block-diag weights."""
    import ml_dtypes
    bf = ml_dtypes.bfloat16
    pm = potentials.max(axis=2, keepdims=True)
    EP = np.exp(KSC * (potentials - pm).astype(np.float64)).astype(bf)  # [B,L,T]
    eps = []
    for c in range(NCORES):
        epc = EP[c * BC:(c + 1) * BC]                    # [32, L, T]
        h = epc.reshape(2, HB, L, T).transpose(0, 3, 2, 1)  # [2, T, L, HB]
        eps.append(np.ascontiguousarray(h.reshape(128, L, HB)))
    expW = np.exp(KSC * trans.astype(np.float64)).astype(np.float32)    # [i, j]
    wbig = np.zeros((128, 128), dtype=bf)
    wbig[0:T, 0:T] = expW.astype(bf)
    wbig[T:128, T:128] = expW.astype(bf)
    scol = np.zeros((128, 2), dtype=bf)
    scol[0:T, 0] = 1
    scol[T:128, 1] = 1
    bmask = np.zeros((2, 128), dtype=bf)
    bmask[0, 0:T] = 1
    bmask[1, T:128] = 1
    return eps, wbig, scol, bmask, expW


def kernel(potentials, lengths, transition_params):
    from concourse.bass_utils import run_bass_kernel_spmd

    potentials = np.ascontiguousarray(np.asarray(potentials, dtype=np.float32))
    lengths = np.asarray(lengths, dtype=np.int32)
    trans = np.ascontiguousarray(np.asarray(transition_params, dtype=np.float32))

    nc = _build_program()
    eps, wbig, scol, bmask, expW = _host_precompute(potentials, trans)
    in_maps = [{"ep": eps[c], "wbig": wbig, "scol": scol, "bmask": bmask}
               for c in range(NCORES)]
    res = run_bass_kernel_spmd(nc, in_maps, core_ids=list(range(NCORES)))
    # [128, L, HB] per core -> EZ [B, L, T]
    parts = []
    for c in range(NCORES):
        arr = res.results[c]["ezhist"]                       # [128, L, HB] bf16
        h = arr.reshape(2, T, L, HB).transpose(0, 3, 2, 1)   # [2, HB, L, T]
        parts.append(h.reshape(BC, L, T))
    EZ = np.concatenate(parts, axis=0).astype(np.float64)    # [B, L, T]

    # Host backtrack in exp domain (monotone-equivalent to max-plus argmax).
    tags = np.zeros((B, L), dtype=np.int64)
    last = EZ[np.arange(B), lengths - 1, :].argmax(axis=1)
    tags[:, L - 1] = last
    lm1 = lengths - 1
    EW = expW.astype(np.float64)
    for t in range(L - 2, -1, -1):
        nxt = tags[:, t + 1]
        cand = EZ[:, t, :] * EW[:, nxt].T
        tags[:, t] = np.where(t >= lm1, last, cand.argmax(axis=1))
    return tags.astype(np.int32)


# revision 21
# speedup vs baseline: 1.4053x; 1.0095x over previous
"""Viterbi CRF decode on 8 Trainium2 NeuronCores — exp-domain PE formulation.

Strategy: data-parallel over batch (32 sequences/core). The forward max-plus DP
runs in the exponential domain so the TensorEngine does the heavy lifting:

    EZ_t[j,b]   = exp(K*(alpha_t[b,j] - n_t[b]))       (n_t arbitrary per-(b,t))
    V[j,b]      = sum_i expW[i,j] * EZ_{t-1}[i,b]      (PE matmul, bf16 in/fp32 acc)
    EZ_t        = V * EP_t                             (DVE elementwise, one op)

where expW = exp(K*trans) and EP_t[j,b] = exp(K*(pot[b,t,j]-max_j pot[b,t,j]))
are host-precomputed input transforms. logsumexp/K approximates max within
log(64)/K; with K=128 the measured error is ~34/262144 tag flips (rel ~1e-4
vs the 2e-2 gate). bf16 quantization of EZ/W/EP adds ~6e-5 alpha noise per
step (products of bf16 are exact in the fp32 PSUM accumulate).

The backtrack only compares values within one (b,t) slice, so any per-(b,t)
rescale of EZ is harmless: argmax_i(alpha[i]+trans[i,j]) == argmax_i
EZ[i]*expW[i,j] (monotone). Every G steps a 1/sum_i EZ rescale (per stream) is
folded off-critical-path into the EP slice of step t+LAG, so the serial chain
is structurally identical every step: one matmul + one DVE multiply.

Device layout: 32 sequences split into two 16-seq streams on partitions 0-63 /
64-127 with BLOCK-DIAGONAL weights [128,128], so ONE matmul per step serves
both streams (K=M=128, N=16), writing one PSUM bank that a single [128,16]
DVE multiply turns into the next bf16 state, written straight into the
history buffer (also the next matmul's rhs). The S-row sums use a [128,2]
selector matmul; the per-b broadcast of 1/S uses a K=2 mask matmul. A one-time
burst of dummy matmuls at startup locks the PE HAM clock-gate at 2.4 GHz
(steady-state gaps are too short to ever re-throttle it).

History streams back to HBM per 128-step chunk; the host backtracks in f64.
"""

import numpy as np

B, L, T = 256, 1024, 64
NCORES = 8
BC = B // NCORES   # 32 sequences per core
HB = BC // 2       # 16 sequences per stream
CH = 128           # steps per DMA chunk
KSC = 128.0        # exp-domain scale
G = 24             # renormalize every G steps
LAG = 4            # renorm scale applied LAG steps after it is measured
NWARM = 40         # startup dummy matmuls to warm the PE HAM clock gate

_cache = {}


def _build_program():
    if "nc" in _cache:
        return _cache["nc"]
    import concourse.bacc as bacc
    import concourse.mybir as mybir
    from concourse.tile import TileContext

    f32 = mybir.dt.float32
    bf16 = mybir.dt.bfloat16

    nc = bacc.Bacc("TRN2", target_bir_lowering=False, debug=False)
    ep_in = nc.dram_tensor("ep", [128, L, HB], bf16, kind="ExternalInput").ap()
    w_in = nc.dram_tensor("wbig", [128, 128], bf16, kind="ExternalInput").ap()
    scol_in = nc.dram_tensor("scol", [128, 2], bf16, kind="ExternalInput").ap()
    bmask_in = nc.dram_tensor("bmask", [2, 128], bf16, kind="ExternalInput").ap()
    hist_out = nc.dram_tensor("ezhist", [128, L, HB], bf16, kind="ExternalOutput").ap()

    with TileContext(nc) as tc:
        with tc.tile_pool(name="const", bufs=1) as cpool, \
             tc.tile_pool(name="ep", bufs=2) as eppool, \
             tc.tile_pool(name="hist", bufs=1) as hpool, \
             tc.tile_pool(name="psv", bufs=4, space="PSUM") as vpool, \
             tc.tile_pool(name="pss", bufs=1, space="PSUM") as spool, \
             tc.tile_pool(name="warm", bufs=1, space="PSUM") as wpool:
            wbig = cpool.tile([128, 128], bf16)
            nc.gpsimd.dma_start(out=wbig[:], in_=w_in[:])
            scol = cpool.tile([128, 2], bf16)
            nc.gpsimd.dma_start(out=scol[:], in_=scol_in[:])
            bmask = cpool.tile([2, 128], bf16)
            nc.gpsimd.dma_start(out=bmask[:], in_=bmask_in[:])
            rs32 = cpool.tile([2, HB], f32)
            rsb = cpool.tile([2, HB], bf16)
            eps1 = cpool.tile([128, HB], bf16)
            hist = hpool.tile([128, L, HB], bf16)   # 32KB/partition

            # One-time PE warmup: ~NWARM back-to-back dummy matmuls (~4-5us of
            # continuous PE activity) flip the HAM clock gate to K=8/8; the
            # per-step gaps afterwards are far below the ~3.4us idle window,
            # so it never re-throttles. A memset source avoids any DMA
            # dependency, so warmup overlaps the input DMAs.
            wsrc = cpool.tile([128, 64], bf16)
            nc.any.memset(wsrc[:], 1.0)
            warm = wpool.tile([64, 64], f32)
            for _ in range(NWARM):
                nc.tensor.matmul(warm[:], wsrc[:], wsrc[:], start=True, stop=True)

            nchunks = L // CH
            for c in range(nchunks):
                ep = eppool.tile([128, CH, HB], bf16, tag="ep")
                nc.gpsimd.dma_start(out=ep[:], in_=ep_in[:, c * CH:(c + 1) * CH, :])

                if c == 0:
                    nc.scalar.copy(hist[:, 0, :], ep[:, 0, :])

                t0 = max(c * CH, 1)
                # Deferred kick tails: emitted 1 / 3 steps after the kick so
                # each op lands in an engine idle gap instead of stalling the
                # chain in the strict per-engine FIFOs.
                recip_at = {}   # t -> sp psum tile
                apply_at = {}   # t -> target ep slot for eps1
                for t in range(t0, (c + 1) * CH):
                    s = t - c * CH
                    vp = vpool.tile([128, HB], f32, tag="v")
                    nc.tensor.matmul(vp[:], wbig[:], hist[:, t - 1, :],
                                     start=True, stop=True)
                    # The renorm scale is pre-folded into eps1 for apply steps,
                    # so the serial chain is identical every step.
                    use_eps1 = t % G == LAG and t >= G + LAG
                    src1 = eps1[:] if use_eps1 else ep[:, s, :]
                    nc.vector.tensor_mul(hist[:, t, :], vp[:], src1)
                    if t in recip_at:
                        sp = recip_at.pop(t)
                        with tc.high_priority(offset=-6):
                            nc.vector.reciprocal(rs32[:], sp[:])
                            nc.scalar.copy(rsb[:], rs32[:])
                    if t in apply_at:
                        tgt = apply_at.pop(t)
                        with tc.high_priority(offset=-3):
                            bcp = spool.tile([128, HB], f32, tag="bc")
                            nc.tensor.matmul(bcp[:], bmask[:], rsb[:],
                                             start=True, stop=True)
                            nc.vector.tensor_mul(eps1[:], ep[:, tgt, :], bcp[:])
                    if t % G == 0 and G <= t < L - LAG:
                        # Off-chain renorm kick: S = per-stream sum_i EZ_t
                        # (rows 0/1 via the selector matmul); 1/S is broadcast
                        # with a K=2 mask matmul and folded into the EP slice
                        # of step t+LAG (always within the same chunk).
                        sp = spool.tile([2, HB], f32, tag="s")
                        nc.tensor.matmul(sp[:], scol[:], hist[:, t, :],
                                         start=True, stop=True)
                        recip_at[t + 1] = sp
                        apply_at[t + 3] = s + LAG

                nc.gpsimd.dma_start(
                    out=hist_out[:, c * CH:(c + 1) * CH, :],
                    in_=hist[:, c * CH:(c + 1) * CH, :],
                )

    nc.compile()
    _cache["nc"] = nc
    return nc


def _host_precompute(potentials, trans):
    """Per-core EP in [128, L, 16] bf16 stream layout + block-diag weights."""
    import ml_dtypes
    bf = ml_dtypes.bfloat16
    pm = potentials.max(axis=2, keepdims=True)
    EP = np.exp(KSC * (potentials - pm).astype(np.float64)).astype(bf)  # [B,L,T]
    eps = []
    for c in range(NCORES):
        epc = EP[c * BC:(c + 1) * BC]                    # [32, L, T]
        h = epc.reshape(2, HB, L, T).transpose(0, 3, 2, 1)  # [2, T, L, HB]
        eps.append(np.ascontiguousarray(h.reshape(128, L, HB)))
    expW = np.exp(KSC * trans.astype(np.float64)).astype(np.float32)    # [i, j]
    wbig = np.zeros((128, 128), dtype=bf)
    wbig[0:T, 0:T] = expW.astype(bf)
    wbig[T:128, T:128] = expW.astype(bf)
    scol = np.zeros((128, 2), dtype=bf)
    scol[0:T, 0] = 1
    scol[T:128, 1] = 1
    bmask = np.zeros((2, 128), dtype=bf)
    bmask[0, 0:T] = 1
    bmask[1, T:128] = 1
    return eps, wbig, scol, bmask, expW


def kernel(potentials, lengths, transition_params):
    from concourse.bass_utils import run_bass_kernel_spmd

    potentials = np.ascontiguousarray(np.asarray(potentials, dtype=np.float32))
    lengths = np.asarray(lengths, dtype=np.int32)
    trans = np.ascontiguousarray(np.asarray(transition_params, dtype=np.float32))

    nc = _build_program()
    eps, wbig, scol, bmask, expW = _host_precompute(potentials, trans)
    in_maps = [{"ep": eps[c], "wbig": wbig, "scol": scol, "bmask": bmask}
               for c in range(NCORES)]
    res = run_bass_kernel_spmd(nc, in_maps, core_ids=list(range(NCORES)))
    # [128, L, HB] per core -> EZ [B, L, T]
    parts = []
    for c in range(NCORES):
        arr = res.results[c]["ezhist"]                       # [128, L, HB] bf16
        h = arr.reshape(2, T, L, HB).transpose(0, 3, 2, 1)   # [2, HB, L, T]
        parts.append(h.reshape(BC, L, T))
    EZ = np.concatenate(parts, axis=0).astype(np.float64)    # [B, L, T]

    # Host backtrack in exp domain (monotone-equivalent to max-plus argmax).
    tags = np.zeros((B, L), dtype=np.int64)
    last = EZ[np.arange(B), lengths - 1, :].argmax(axis=1)
    tags[:, L - 1] = last
    lm1 = lengths - 1
    EW = expW.astype(np.float64)
    for t in range(L - 2, -1, -1):
        nxt = tags[:, t + 1]
        cand = EZ[:, t, :] * EW[:, nxt].T
        tags[:, t] = np.where(t >= lm1, last, cand.argmax(axis=1))
    return tags.astype(np.int32)


# revision 23
# speedup vs baseline: 1.4117x; 1.0045x over previous
"""Viterbi CRF decode on 8 Trainium2 NeuronCores — exp-domain PE formulation.

Strategy: data-parallel over batch (32 sequences/core). The forward max-plus DP
runs in the exponential domain so the TensorEngine does the heavy lifting:

    EZ_t[j,b]   = exp(K*(alpha_t[b,j] - n_t[b]))       (n_t arbitrary per-(b,t))
    V[j,b]      = sum_i expW[i,j] * EZ_{t-1}[i,b]      (PE matmul, bf16 in/fp32 acc)
    EZ_t        = V * EP_t                             (DVE elementwise, one op)

where expW = exp(K*trans) and EP_t[j,b] = exp(K*(pot[b,t,j]-max_j pot[b,t,j]))
are host-precomputed input transforms. logsumexp/K approximates max within
log(64)/K; with K=128 the measured error is ~34/262144 tag flips (rel ~1e-4
vs the 2e-2 gate). bf16 quantization of EZ/W/EP adds ~6e-5 alpha noise per
step (products of bf16 are exact in the fp32 PSUM accumulate).

The backtrack only compares values within one (b,t) slice, so any per-(b,t)
rescale of EZ is harmless: argmax_i(alpha[i]+trans[i,j]) == argmax_i
EZ[i]*expW[i,j] (monotone). Every G steps a 1/sum_i EZ rescale (per stream) is
folded off-critical-path into the EP slice of step t+LAG, so the serial chain
is structurally identical every step: one matmul + one DVE multiply.

Device layout: 32 sequences split into two 16-seq streams on partitions 0-63 /
64-127 with BLOCK-DIAGONAL weights [128,128], so ONE matmul per step serves
both streams (K=M=128, N=16), writing one PSUM bank that a single [128,16]
DVE multiply turns into the next bf16 state, written straight into the
history buffer (also the next matmul's rhs). The S-row sums use a [128,2]
selector matmul; the per-b broadcast of 1/S uses a K=2 mask matmul. A one-time
burst of dummy matmuls at startup locks the PE HAM clock-gate at 2.4 GHz
(steady-state gaps are too short to ever re-throttle it).

History streams back to HBM per 128-step chunk; the host backtracks in f64.
"""

import numpy as np

B, L, T = 256, 1024, 64
NCORES = 8
BC = B // NCORES   # 32 sequences per core
HB = BC // 2       # 16 sequences per stream
CH = 128           # steps per DMA chunk
KSC = 128.0        # exp-domain scale
G = 24             # renormalize every G steps
LAG = 4            # renorm scale applied LAG steps after it is measured
NWARM = 40         # startup dummy matmuls to warm the PE HAM clock gate

_cache = {}


def _build_program():
    if "nc" in _cache:
        return _cache["nc"]
    import concourse.bacc as bacc
    import concourse.mybir as mybir
    from concourse.tile import TileContext

    f32 = mybir.dt.float32
    bf16 = mybir.dt.bfloat16

    nc = bacc.Bacc("TRN2", target_bir_lowering=False, debug=False)
    ep_in = nc.dram_tensor("ep", [128, L, HB], bf16, kind="ExternalInput").ap()
    w_in = nc.dram_tensor("wbig", [128, 128], bf16, kind="ExternalInput").ap()
    scol_in = nc.dram_tensor("scol", [128, 2], bf16, kind="ExternalInput").ap()
    bmask_in = nc.dram_tensor("bmask", [2, 128], bf16, kind="ExternalInput").ap()
    hist_out = nc.dram_tensor("ezhist", [128, L, HB], bf16, kind="ExternalOutput").ap()

    with TileContext(nc) as tc:
        with tc.tile_pool(name="const", bufs=1) as cpool, \
             tc.tile_pool(name="ep", bufs=2) as eppool, \
             tc.tile_pool(name="hist", bufs=1) as hpool, \
             tc.tile_pool(name="psv", bufs=4, space="PSUM") as vpool, \
             tc.tile_pool(name="pss", bufs=1, space="PSUM") as spool, \
             tc.tile_pool(name="warm", bufs=1, space="PSUM") as wpool:
            wbig = cpool.tile([128, 128], bf16)
            nc.gpsimd.dma_start(out=wbig[:], in_=w_in[:])
            scol = cpool.tile([128, 2], bf16)
            nc.gpsimd.dma_start(out=scol[:], in_=scol_in[:])
            bmask = cpool.tile([2, 128], bf16)
            nc.gpsimd.dma_start(out=bmask[:], in_=bmask_in[:])
            rs32 = cpool.tile([2, HB], f32)
            rsb = cpool.tile([2, HB], bf16)
            eps1 = cpool.tile([128, HB], bf16)
            hist = hpool.tile([128, L, HB], bf16)   # 32KB/partition

            # One-time PE warmup: ~NWARM back-to-back dummy matmuls (~4-5us of
            # continuous PE activity) flip the HAM clock gate to K=8/8; the
            # per-step gaps afterwards are far below the ~3.4us idle window,
            # so it never re-throttles. A memset source avoids any DMA
            # dependency, so warmup overlaps the input DMAs.
            wsrc = cpool.tile([128, 64], bf16)
            nc.any.memset(wsrc[:], 1.0)
            warm = wpool.tile([64, 64], f32)
            for _ in range(NWARM):
                nc.tensor.matmul(warm[:], wsrc[:], wsrc[:], start=True, stop=True)

            nchunks = L // CH
            for c in range(nchunks):
                ep = eppool.tile([128, CH, HB], bf16, tag="ep")
                nc.gpsimd.dma_start(out=ep[:], in_=ep_in[:, c * CH:(c + 1) * CH, :])

                if c == 0:
                    # DVE copy: keeps the ScalarE ACT table load (~1.6us) off
                    # the critical startup path.
                    nc.vector.tensor_copy(hist[:, 0, :], ep[:, 0, :])

                t0 = max(c * CH, 1)
                # Deferred kick tails: emitted 1 / 3 steps after the kick so
                # each op lands in an engine idle gap instead of stalling the
                # chain in the strict per-engine FIFOs.
                recip_at = {}   # t -> sp psum tile
                apply_at = {}   # t -> target ep slot for eps1
                for t in range(t0, (c + 1) * CH):
                    s = t - c * CH
                    vp = vpool.tile([128, HB], f32, tag="v")
                    nc.tensor.matmul(vp[:], wbig[:], hist[:, t - 1, :],
                                     start=True, stop=True)
                    # The renorm scale is pre-folded into eps1 for apply steps,
                    # so the serial chain is identical every step.
                    use_eps1 = t % G == LAG and t >= G + LAG
                    src1 = eps1[:] if use_eps1 else ep[:, s, :]
                    nc.vector.tensor_mul(hist[:, t, :], vp[:], src1)
                    if t in recip_at:
                        sp = recip_at.pop(t)
                        with tc.high_priority(offset=-12):
                            nc.vector.reciprocal(rs32[:], sp[:])
                            nc.scalar.copy(rsb[:], rs32[:])
                    if t in apply_at:
                        tgt = apply_at.pop(t)
                        with tc.high_priority(offset=-8):
                            bcp = spool.tile([128, HB], f32, tag="bc")
                            nc.tensor.matmul(bcp[:], bmask[:], rsb[:],
                                             start=True, stop=True)
                            nc.vector.tensor_mul(eps1[:], ep[:, tgt, :], bcp[:])
                    if t % G == 0 and G <= t < L - LAG:
                        # Off-chain renorm kick: S = per-stream sum_i EZ_t
                        # (rows 0/1 via the selector matmul); 1/S is broadcast
                        # with a K=2 mask matmul and folded into the EP slice
                        # of step t+LAG (always within the same chunk).
                        sp = spool.tile([2, HB], f32, tag="s")
                        nc.tensor.matmul(sp[:], scol[:], hist[:, t, :],
                                         start=True, stop=True)
                        recip_at[t + 1] = sp
                        apply_at[t + 3] = s + LAG

                nc.gpsimd.dma_start(
                    out=hist_out[:, c * CH:(c + 1) * CH, :],
                    in_=hist[:, c * CH:(c + 1) * CH, :],
                )

    nc.compile()
    _cache["nc"] = nc
    return nc


def _host_precompute(potentials, trans):
    """Per-core EP in [128, L, 16] bf16 stream layout + block-diag weights."""
    import ml_dtypes
    bf = ml_dtypes.bfloat16
    pm = potentials.max(axis=2, keepdims=True)
    EP = np.exp(KSC * (potentials - pm).astype(np.float64)).astype(bf)  # [B,L,T]
    eps = []
    for c in range(NCORES):
        epc = EP[c * BC:(c + 1) * BC]                    # [32, L, T]
        h = epc.reshape(2, HB, L, T).transpose(0, 3, 2, 1)  # [2, T, L, HB]
        eps.append(np.ascontiguousarray(h.reshape(128, L, HB)))
    expW = np.exp(KSC * trans.astype(np.float64)).astype(np.float32)    # [i, j]
    wbig = np.zeros((128, 128), dtype=bf)
    wbig[0:T, 0:T] = expW.astype(bf)
    wbig[T:128, T:128] = expW.astype(bf)
    scol = np.zeros((128, 2), dtype=bf)
    scol[0:T, 0] = 1
    scol[T:128, 1] = 1
    bmask = np.zeros((2, 128), dtype=bf)
    bmask[0, 0:T] = 1
    bmask[1, T:128] = 1
    return eps, wbig, scol, bmask, expW


def kernel(potentials, lengths, transition_params):
    from concourse.bass_utils import run_bass_kernel_spmd

    potentials = np.ascontiguousarray(np.asarray(potentials, dtype=np.float32))
    lengths = np.asarray(lengths, dtype=np.int32)
    trans = np.ascontiguousarray(np.asarray(transition_params, dtype=np.float32))

    nc = _build_program()
    eps, wbig, scol, bmask, expW = _host_precompute(potentials, trans)
    in_maps = [{"ep": eps[c], "wbig": wbig, "scol": scol, "bmask": bmask}
               for c in range(NCORES)]
    res = run_bass_kernel_spmd(nc, in_maps, core_ids=list(range(NCORES)))
    # [128, L, HB] per core -> EZ [B, L, T]
    parts = []
    for c in range(NCORES):
        arr = res.results[c]["ezhist"]                       # [128, L, HB] bf16
        h = arr.reshape(2, T, L, HB).transpose(0, 3, 2, 1)   # [2, HB, L, T]
        parts.append(h.reshape(BC, L, T))
    EZ = np.concatenate(parts, axis=0).astype(np.float64)    # [B, L, T]

    # Host backtrack in exp domain (monotone-equivalent to max-plus argmax).
    tags = np.zeros((B, L), dtype=np.int64)
    last = EZ[np.arange(B), lengths - 1, :].argmax(axis=1)
    tags[:, L - 1] = last
    lm1 = lengths - 1
    EW = expW.astype(np.float64)
    for t in range(L - 2, -1, -1):
        nxt = tags[:, t + 1]
        cand = EZ[:, t, :] * EW[:, nxt].T
        tags[:, t] = np.where(t >= lm1, last, cand.argmax(axis=1))
    return tags.astype(np.int32)


# revision 28
# speedup vs baseline: 1.4130x; 1.0009x over previous
"""Viterbi CRF decode on 8 Trainium2 NeuronCores — exp-domain PE formulation.

Strategy: data-parallel over batch (32 sequences/core). The forward max-plus DP
runs in the exponential domain so the TensorEngine does the heavy lifting:

    EZ_t[j,b]   = exp(K*(alpha_t[b,j] - n_t[b]))       (n_t arbitrary per-(b,t))
    V[j,b]      = sum_i expW[i,j] * EZ_{t-1}[i,b]      (PE matmul, bf16 in/fp32 acc)
    EZ_t        = V * EP_t                             (DVE elementwise, one op)

where expW = exp(K*trans) and EP_t[j,b] = exp(K*(pot[b,t,j]-max_j pot[b,t,j]))
are host-precomputed input transforms. logsumexp/K approximates max within
log(64)/K; with K=128 the measured error is ~34/262144 tag flips (rel ~1e-4
vs the 2e-2 gate). bf16 quantization of EZ/W/EP adds ~6e-5 alpha noise per
step (products of bf16 are exact in the fp32 PSUM accumulate).

The backtrack only compares values within one (b,t) slice, so any per-(b,t)
rescale of EZ is harmless: argmax_i(alpha[i]+trans[i,j]) == argmax_i
EZ[i]*expW[i,j] (monotone). Every G steps a 1/sum_i EZ rescale (per stream) is
folded off-critical-path into the EP slice of step t+LAG, so the serial chain
is structurally identical every step: one matmul + one DVE multiply.

Device layout: 32 sequences split into two 16-seq streams on partitions 0-63 /
64-127 with BLOCK-DIAGONAL weights [128,128], so ONE matmul per step serves
both streams (K=M=128, N=16), writing one PSUM bank that a single [128,16]
DVE multiply turns into the next bf16 state, written straight into the
history buffer (also the next matmul's rhs). The S-row sums use a [128,2]
selector matmul; the per-b broadcast of 1/S uses a K=2 mask matmul. A one-time
burst of dummy matmuls at startup locks the PE HAM clock-gate at 2.4 GHz
(steady-state gaps are too short to ever re-throttle it).

History streams back to HBM per 128-step chunk; the host backtracks in f64.
"""

import numpy as np

B, L, T = 256, 1024, 64
NCORES = 8
BC = B // NCORES   # 32 sequences per core
HB = BC // 2       # 16 sequences per stream
CH = 128           # steps per DMA chunk
KSC = 128.0        # exp-domain scale
G = 24             # renormalize every G steps
LAG = 4            # renorm scale applied LAG steps after it is measured
NWARM = 40         # startup dummy matmuls to warm the PE HAM clock gate

_cache = {}


def _build_program():
    if "nc" in _cache:
        return _cache["nc"]
    import concourse.bacc as bacc
    import concourse.mybir as mybir
    from concourse.tile import TileContext

    f32 = mybir.dt.float32
    bf16 = mybir.dt.bfloat16

    nc = bacc.Bacc("TRN2", target_bir_lowering=False, debug=False)
    ep_in = nc.dram_tensor("ep", [128, L, HB], bf16, kind="ExternalInput").ap()
    w_in = nc.dram_tensor("wbig", [128, 128], bf16, kind="ExternalInput").ap()
    scol_in = nc.dram_tensor("scol", [128, 2], bf16, kind="ExternalInput").ap()
    bmask_in = nc.dram_tensor("bmask", [2, 128], bf16, kind="ExternalInput").ap()
    hist_out = nc.dram_tensor("ezhist", [128, L, HB], bf16, kind="ExternalOutput").ap()

    with TileContext(nc) as tc:
        with tc.tile_pool(name="const", bufs=1) as cpool, \
             tc.tile_pool(name="ep", bufs=2) as eppool, \
             tc.tile_pool(name="hist", bufs=1) as hpool, \
             tc.tile_pool(name="psv", bufs=4, space="PSUM") as vpool, \
             tc.tile_pool(name="pss", bufs=1, space="PSUM") as spool, \
             tc.tile_pool(name="warm", bufs=1, space="PSUM") as wpool:
            # ep chunk 0 first: its first 16 steps are the only thing gating
            # the chain start, so that DMA goes ahead of everything else.
            ep0 = eppool.tile([128, CH, HB], bf16, tag="ep")
            nc.gpsimd.dma_start(out=ep0[:, 0:16, :], in_=ep_in[:, 0:16, :])
            wbig = cpool.tile([128, 128], bf16)
            nc.gpsimd.dma_start(out=wbig[:], in_=w_in[:])
            scol = cpool.tile([128, 2], bf16)
            nc.gpsimd.dma_start(out=scol[:], in_=scol_in[:])
            bmask = cpool.tile([2, 128], bf16)
            nc.gpsimd.dma_start(out=bmask[:], in_=bmask_in[:])
            nc.gpsimd.dma_start(out=ep0[:, 16:CH, :], in_=ep_in[:, 16:CH, :])
            sps = cpool.tile([2, HB], f32)
            ones2 = cpool.tile([2, HB], f32)
            nc.any.memset(ones2[:], 1.0)
            rs32 = cpool.tile([2, HB], f32)
            rsb = cpool.tile([2, HB], bf16)
            eps1 = cpool.tile([128, HB], bf16)
            hist = hpool.tile([128, L, HB], bf16)   # 32KB/partition

            # One-time PE warmup: ~NWARM back-to-back dummy matmuls (~4-5us of
            # continuous PE activity) flip the HAM clock gate to K=8/8; the
            # per-step gaps afterwards are far below the ~3.4us idle window,
            # so it never re-throttles. A memset source avoids any DMA
            # dependency, so warmup overlaps the input DMAs.
            wsrc = cpool.tile([128, 64], bf16)
            nc.any.memset(wsrc[:], 1.0)
            warm = wpool.tile([64, 64], f32)
            for _ in range(NWARM):
                nc.tensor.matmul(warm[:], wsrc[:], wsrc[:], start=True, stop=True)

            nchunks = L // CH
            for c in range(nchunks):
                if c == 0:
                    ep = ep0
                else:
                    ep = eppool.tile([128, CH, HB], bf16, tag="ep")
                    nc.gpsimd.dma_start(out=ep[:],
                                        in_=ep_in[:, c * CH:(c + 1) * CH, :])

                if c == 0:
                    # DVE copy: keeps the ScalarE ACT table load (~1.6us) off
                    # the critical startup path.
                    nc.vector.tensor_copy(hist[:, 0, :], ep[:, 0, :])

                t0 = max(c * CH, 1)
                # Deferred kick tails: emitted 1 / 3 steps after the kick so
                # each op lands in an engine idle gap instead of stalling the
                # chain in the strict per-engine FIFOs.
                recip_at = {}   # t -> sp psum tile
                apply_at = {}   # t -> target ep slot for eps1
                for t in range(t0, (c + 1) * CH):
                    s = t - c * CH
                    vp = vpool.tile([128, HB], f32, tag="v")
                    nc.tensor.matmul(vp[:], wbig[:], hist[:, t - 1, :],
                                     start=True, stop=True)
                    # The renorm scale is pre-folded into eps1 for apply steps,
                    # so the serial chain is identical every step.
                    use_eps1 = t % G == LAG and t >= G + LAG
                    src1 = eps1[:] if use_eps1 else ep[:, s, :]
                    nc.vector.tensor_mul(hist[:, t, :], vp[:], src1)
                    if t in recip_at:
                        sp = recip_at.pop(t)
                        with tc.high_priority(offset=-12):
                            nc.vector.reciprocal(rs32[:], sp[:])
                            nc.scalar.copy(rsb[:], rs32[:])
                    if t in apply_at:
                        tgt = apply_at.pop(t)
                        with tc.high_priority(offset=-8):
                            bcp = spool.tile([128, HB], f32, tag="bc")
                            nc.tensor.matmul(bcp[:], bmask[:], rsb[:],
                                             start=True, stop=True)
                            nc.vector.tensor_mul(eps1[:], ep[:, tgt, :], bcp[:])
                    if t % G == 0 and G <= t < L - LAG:
                        # Off-chain renorm kick: S = per-stream sum_i EZ_t
                        # (rows 0/1 via the selector matmul); 1/S is broadcast
                        # with a K=2 mask matmul and folded into the EP slice
                        # of step t+LAG (always within the same chunk).
                        sp = spool.tile([2, HB], f32, tag="s")
                        nc.tensor.matmul(sp[:], scol[:], hist[:, t, :],
                                         start=True, stop=True)
                        recip_at[t + 1] = sp
                        apply_at[t + 3] = s + LAG

                # Split the history write-back so the bulk of each chunk (and
                # especially the final one) overlaps the still-running chain;
                # only the last 16 steps' 64KB remain for the tail.
                nc.gpsimd.dma_start(
                    out=hist_out[:, c * CH:c * CH + 112, :],
                    in_=hist[:, c * CH:c * CH + 112, :],
                )
                nc.gpsimd.dma_start(
                    out=hist_out[:, c * CH + 112:(c + 1) * CH, :],
                    in_=hist[:, c * CH + 112:(c + 1) * CH, :],
                )

    nc.compile()
    _cache["nc"] = nc
    return nc


def _host_precompute(potentials, trans):
    """Per-core EP in [128, L, 16] bf16 stream layout + block-diag weights."""
    import ml_dtypes
    bf = ml_dtypes.bfloat16
    pm = potentials.max(axis=2, keepdims=True)
    EP = np.exp(KSC * (potentials - pm).astype(np.float64)).astype(bf)  # [B,L,T]
    eps = []
    for c in range(NCORES):
        epc = EP[c * BC:(c + 1) * BC]                    # [32, L, T]
        h = epc.reshape(2, HB, L, T).transpose(0, 3, 2, 1)  # [2, T, L, HB]
        eps.append(np.ascontiguousarray(h.reshape(128, L, HB)))
    expW = np.exp(KSC * trans.astype(np.float64)).astype(np.float32)    # [i, j]
    wbig = np.zeros((128, 128), dtype=bf)
    wbig[0:T, 0:T] = expW.astype(bf)
    wbig[T:128, T:128] = expW.astype(bf)
    scol = np.zeros((128, 2), dtype=bf)
    scol[0:T, 0] = 1
    scol[T:128, 1] = 1
    bmask = np.zeros((2, 128), dtype=bf)
    bmask[0, 0:T] = 1
    bmask[1, T:128] = 1
    return eps, wbig, scol, bmask, expW


def kernel(potentials, lengths, transition_params):
    from concourse.bass_utils import run_bass_kernel_spmd

    potentials = np.ascontiguousarray(np.asarray(potentials, dtype=np.float32))
    lengths = np.asarray(lengths, dtype=np.int32)
    trans = np.ascontiguousarray(np.asarray(transition_params, dtype=np.float32))

    nc = _build_program()
    eps, wbig, scol, bmask, expW = _host_precompute(potentials, trans)
    in_maps = [{"ep": eps[c], "wbig": wbig, "scol": scol, "bmask": bmask}
               for c in range(NCORES)]
    res = run_bass_kernel_spmd(nc, in_maps, core_ids=list(range(NCORES)))
    # [128, L, HB] per core -> EZ [B, L, T]
    parts = []
    for c in range(NCORES):
        arr = res.results[c]["ezhist"]                       # [128, L, HB] bf16
        h = arr.reshape(2, T, L, HB).transpose(0, 3, 2, 1)   # [2, HB, L, T]
        parts.append(h.reshape(BC, L, T))
    EZ = np.concatenate(parts, axis=0).astype(np.float64)    # [B, L, T]

    # Host backtrack in exp domain (monotone-equivalent to max-plus argmax).
    tags = np.zeros((B, L), dtype=np.int64)
    last = EZ[np.arange(B), lengths - 1, :].argmax(axis=1)
    tags[:, L - 1] = last
    lm1 = lengths - 1
    EW = expW.astype(np.float64)
    for t in range(L - 2, -1, -1):
        nxt = tags[:, t + 1]
        cand = EZ[:, t, :] * EW[:, nxt].T
        tags[:, t] = np.where(t >= lm1, last, cand.argmax(axis=1))
    return tags.astype(np.int32)


# revision 31
# speedup vs baseline: 1.4161x; 1.0022x over previous
"""Viterbi CRF decode on 8 Trainium2 NeuronCores — exp-domain PE formulation.

Strategy: data-parallel over batch (32 sequences/core). The forward max-plus DP
runs in the exponential domain so the TensorEngine does the heavy lifting:

    EZ_t[j,b]   = exp(K*(alpha_t[b,j] - n_t[b]))       (n_t arbitrary per-(b,t))
    V[j,b]      = sum_i expW[i,j] * EZ_{t-1}[i,b]      (PE matmul, bf16 in/fp32 acc)
    EZ_t        = V * EP_t                             (DVE elementwise, one op)

where expW = exp(K*trans) and EP_t[j,b] = exp(K*(pot[b,t,j]-max_j pot[b,t,j]))
are host-precomputed input transforms. logsumexp/K approximates max within
log(64)/K; with K=128 the measured error is ~34/262144 tag flips (rel ~1e-4
vs the 2e-2 gate). bf16 quantization of EZ/W/EP adds ~6e-5 alpha noise per
step (products of bf16 are exact in the fp32 PSUM accumulate).

The backtrack only compares values within one (b,t) slice, so any per-(b,t)
rescale of EZ is harmless: argmax_i(alpha[i]+trans[i,j]) == argmax_i
EZ[i]*expW[i,j] (monotone). Every G steps a 1/sum_i EZ rescale (per stream) is
folded off-critical-path into the EP slice of step t+LAG, so the serial chain
is structurally identical every step: one matmul + one DVE multiply.

Device layout: 32 sequences split into two 16-seq streams on partitions 0-63 /
64-127 with BLOCK-DIAGONAL weights [128,128], so ONE matmul per step serves
both streams (K=M=128, N=16), writing one PSUM bank that a single [128,16]
DVE multiply turns into the next bf16 state, written straight into the
history buffer (also the next matmul's rhs). The S-row sums use a [128,2]
selector matmul; the per-b broadcast of 1/S uses a K=2 mask matmul. A one-time
burst of dummy matmuls at startup locks the PE HAM clock-gate at 2.4 GHz
(steady-state gaps are too short to ever re-throttle it).

History streams back to HBM per 128-step chunk; the host backtracks in f64.
"""

import numpy as np

B, L, T = 256, 1024, 64
NCORES = 8
BC = B // NCORES   # 32 sequences per core
HB = BC // 2       # 16 sequences per stream
CH = 128           # steps per DMA chunk
KSC = 128.0        # exp-domain scale
G = 24             # renormalize every G steps
LAG = 4            # renorm scale applied LAG steps after it is measured
NWARM = 40         # startup dummy matmuls to warm the PE HAM clock gate

_cache = {}


def _build_program():
    if "nc" in _cache:
        return _cache["nc"]
    import concourse.bacc as bacc
    import concourse.mybir as mybir
    from concourse.tile import TileContext

    f32 = mybir.dt.float32
    bf16 = mybir.dt.bfloat16

    nc = bacc.Bacc("TRN2", target_bir_lowering=False, debug=False)
    ep_in = nc.dram_tensor("ep", [128, L, HB], bf16, kind="ExternalInput").ap()
    w_in = nc.dram_tensor("wbig", [128, 128], bf16, kind="ExternalInput").ap()
    scol_in = nc.dram_tensor("scol", [128, 2], bf16, kind="ExternalInput").ap()
    bmask_in = nc.dram_tensor("bmask", [2, 128], bf16, kind="ExternalInput").ap()
    hist_out = nc.dram_tensor("ezhist", [128, L, HB], bf16, kind="ExternalOutput").ap()

    with TileContext(nc) as tc:
        with tc.tile_pool(name="const", bufs=1) as cpool, \
             tc.tile_pool(name="ep", bufs=2) as eppool, \
             tc.tile_pool(name="hist", bufs=1) as hpool, \
             tc.tile_pool(name="psv", bufs=4, space="PSUM") as vpool, \
             tc.tile_pool(name="pss", bufs=1, space="PSUM") as spool, \
             tc.tile_pool(name="warm", bufs=1, space="PSUM") as wpool:
            # ep chunk 0 first: its first 16 steps are the only thing gating
            # the chain start, so that DMA goes ahead of everything else.
            ep0 = eppool.tile([128, CH, HB], bf16, tag="ep")
            nc.gpsimd.dma_start(out=ep0[:, 0:16, :], in_=ep_in[:, 0:16, :])
            wbig = cpool.tile([128, 128], bf16)
            nc.gpsimd.dma_start(out=wbig[:], in_=w_in[:])
            scol = cpool.tile([128, 2], bf16)
            nc.gpsimd.dma_start(out=scol[:], in_=scol_in[:])
            bmask = cpool.tile([2, 128], bf16)
            nc.gpsimd.dma_start(out=bmask[:], in_=bmask_in[:])
            nc.gpsimd.dma_start(out=ep0[:, 16:CH, :], in_=ep_in[:, 16:CH, :])
            sps = cpool.tile([2, HB], f32)
            ones2 = cpool.tile([2, HB], f32)
            nc.any.memset(ones2[:], 1.0)
            rs32 = cpool.tile([2, HB], f32)
            rsb = cpool.tile([2, HB], bf16)
            eps1 = cpool.tile([128, HB], bf16)
            hist = hpool.tile([128, L, HB], bf16)   # 32KB/partition

            # One-time PE warmup: ~NWARM back-to-back dummy matmuls (~4-5us of
            # continuous PE activity) flip the HAM clock gate to K=8/8; the
            # per-step gaps afterwards are far below the ~3.4us idle window,
            # so it never re-throttles. A memset source avoids any DMA
            # dependency, so warmup overlaps the input DMAs.
            wsrc = cpool.tile([128, 64], bf16)
            nc.any.memset(wsrc[:], 1.0)
            warm = wpool.tile([64, 64], f32)
            for _ in range(NWARM):
                nc.tensor.matmul(warm[:], wsrc[:], wsrc[:], start=True, stop=True)

            nchunks = L // CH
            for c in range(nchunks):
                if c == 0:
                    ep = ep0
                else:
                    ep = eppool.tile([128, CH, HB], bf16, tag="ep")
                    nc.gpsimd.dma_start(out=ep[:],
                                        in_=ep_in[:, c * CH:(c + 1) * CH, :])

                if c == 0:
                    # Off-chain: the chain's first matmul reads ep directly,
                    # so this copy (backtrack needs t=0 in the history) can
                    # happen on ScalarE any time before the chunk write-back.
                    nc.scalar.copy(hist[:, 0, :], ep[:, 0, :])

                t0 = max(c * CH, 1)
                # Deferred kick tails: emitted 1 / 3 steps after the kick so
                # each op lands in an engine idle gap instead of stalling the
                # chain in the strict per-engine FIFOs.
                recip_at = {}   # t -> sp psum tile
                apply_at = {}   # t -> target ep slot for eps1
                for t in range(t0, (c + 1) * CH):
                    s = t - c * CH
                    vp = vpool.tile([128, HB], f32, tag="v")
                    rhs = ep[:, 0, :] if t == 1 else hist[:, t - 1, :]
                    nc.tensor.matmul(vp[:], wbig[:], rhs, start=True, stop=True)
                    # The renorm scale is pre-folded into eps1 for apply steps,
                    # so the serial chain is identical every step.
                    use_eps1 = t % G == LAG and t >= G + LAG
                    src1 = eps1[:] if use_eps1 else ep[:, s, :]
                    nc.vector.tensor_mul(hist[:, t, :], vp[:], src1)
                    if t in recip_at:
                        sp = recip_at.pop(t)
                        with tc.high_priority(offset=-12):
                            nc.vector.reciprocal(rs32[:], sp[:])
                            nc.scalar.copy(rsb[:], rs32[:])
                    if t in apply_at:
                        tgt = apply_at.pop(t)
                        with tc.high_priority(offset=-8):
                            bcp = spool.tile([128, HB], f32, tag="bc")
                            nc.tensor.matmul(bcp[:], bmask[:], rsb[:],
                                             start=True, stop=True)
                            nc.vector.tensor_mul(eps1[:], ep[:, tgt, :], bcp[:])
                    if t % G == 0 and G <= t < L - LAG:
                        # Off-chain renorm kick: S = per-stream sum_i EZ_t
                        # (rows 0/1 via the selector matmul); 1/S is broadcast
                        # with a K=2 mask matmul and folded into the EP slice
                        # of step t+LAG (always within the same chunk).
                        sp = spool.tile([2, HB], f32, tag="s")
                        nc.tensor.matmul(sp[:], scol[:], hist[:, t, :],
                                         start=True, stop=True)
                        recip_at[t + 1] = sp
                        apply_at[t + 3] = s + LAG

                if c < nchunks - 1:
                    nc.gpsimd.dma_start(
                        out=hist_out[:, c * CH:(c + 1) * CH, :],
                        in_=hist[:, c * CH:(c + 1) * CH, :],
                    )
                else:
                    # Split only the final write-back so the bulk overlaps the
                    # still-running chain and just 64KB remains for the tail.
                    nc.gpsimd.dma_start(
                        out=hist_out[:, c * CH:c * CH + 112, :],
                        in_=hist[:, c * CH:c * CH + 112, :],
                    )
                    nc.gpsimd.dma_start(
                        out=hist_out[:, c * CH + 112:(c + 1) * CH, :],
                        in_=hist[:, c * CH + 112:(c + 1) * CH, :],
                    )

    nc.compile()
    _cache["nc"] = nc
    return nc


def _host_precompute(potentials, trans):
    """Per-core EP in [128, L, 16] bf16 stream layout + block-diag weights."""
    import ml_dtypes
    bf = ml_dtypes.bfloat16
    pm = potentials.max(axis=2, keepdims=True)
    EP = np.exp(KSC * (potentials - pm).astype(np.float64)).astype(bf)  # [B,L,T]
    eps = []
    for c in range(NCORES):
        epc = EP[c * BC:(c + 1) * BC]                    # [32, L, T]
        h = epc.reshape(2, HB, L, T).transpose(0, 3, 2, 1)  # [2, T, L, HB]
        eps.append(np.ascontiguousarray(h.reshape(128, L, HB)))
    expW = np.exp(KSC * trans.astype(np.float64)).astype(np.float32)    # [i, j]
    wbig = np.zeros((128, 128), dtype=bf)
    wbig[0:T, 0:T] = expW.astype(bf)
    wbig[T:128, T:128] = expW.astype(bf)
    scol = np.zeros((128, 2), dtype=bf)
    scol[0:T, 0] = 1
    scol[T:128, 1] = 1
    bmask = np.zeros((2, 128), dtype=bf)
    bmask[0, 0:T] = 1
    bmask[1, T:128] = 1
    return eps, wbig, scol, bmask, expW


def kernel(potentials, lengths, transition_params):
    from concourse.bass_utils import run_bass_kernel_spmd

    potentials = np.ascontiguousarray(np.asarray(potentials, dtype=np.float32))
    lengths = np.asarray(lengths, dtype=np.int32)
    trans = np.ascontiguousarray(np.asarray(transition_params, dtype=np.float32))

    nc = _build_program()
    eps, wbig, scol, bmask, expW = _host_precompute(potentials, trans)
    in_maps = [{"ep": eps[c], "wbig": wbig, "scol": scol, "bmask": bmask}
               for c in range(NCORES)]
    res = run_bass_kernel_spmd(nc, in_maps, core_ids=list(range(NCORES)))
    # [128, L, HB] per core -> EZ [B, L, T]
    parts = []
    for c in range(NCORES):
        arr = res.results[c]["ezhist"]                       # [128, L, HB] bf16
        h = arr.reshape(2, T, L, HB).transpose(0, 3, 2, 1)   # [2, HB, L, T]
        parts.append(h.reshape(BC, L, T))
    EZ = np.concatenate(parts, axis=0).astype(np.float64)    # [B, L, T]

    # Host backtrack in exp domain (monotone-equivalent to max-plus argmax).
    tags = np.zeros((B, L), dtype=np.int64)
    last = EZ[np.arange(B), lengths - 1, :].argmax(axis=1)
    tags[:, L - 1] = last
    lm1 = lengths - 1
    EW = expW.astype(np.float64)
    for t in range(L - 2, -1, -1):
        nxt = tags[:, t + 1]
        cand = EZ[:, t, :] * EW[:, nxt].T
        tags[:, t] = np.where(t >= lm1, last, cand.argmax(axis=1))
    return tags.astype(np.int32)
